# revision 1
# baseline (speedup 1.0000x reference)
"""InfoNCE loss kernel for Trainium2, 8 NeuronCores.

Reference computation:
    z = l2_normalize(concat([polyline_embs, c_embs]))   # [8192, 512]
    sim = z @ z.T                                       # [8192, 8192]
    denom_i = sum_{j != i} exp(sim_ij / T)
    pos_i   = sim[i, i +- B]
    loss    = mean(log(denom_i) - pos_i / T)

Sharding: row-block parallel. Core k computes rows [k*1024, (k+1)*1024) of sim
against all 8192 columns (the "all-gather" is free: every core receives the
full transposed embedding matrix as input). The scalar loss is reduced on host
(the "all-reduce").

Each core runs an identical program on different input slices (SPMD without
partition-id): per-core inputs are pre-sliced on host.
"""

import os

import numpy as np
import ml_dtypes

B = 4096
D = 512
N = 2 * B            # 8192 rows of sim
NCORES = 8
RPC = N // NCORES    # 1024 rows per core
P = 128              # partitions
ITILES = RPC // P    # 8 row tiles per core
CTILES = D // P      # 4 contraction chunks of 128
NT = 512             # column-tile width
NTILES = N // NT     # 16 column tiles
INV_T = 2.0          # 1 / temperature

_CACHE = {}


def _build_bass():
    """Trace the per-core Bass program (identical for all 8 cores)."""
    import concourse.bass as bass
    import concourse.tile as tile
    from concourse import bacc, mybir

    dt = mybir.dt
    AF = mybir.ActivationFunctionType
    ALU = mybir.AluOpType

    nc = bacc.Bacc(None, target_bir_lowering=False, debug=False, num_swdge_queues=4)

    # -------- DRAM I/O (host-pretiled, bf16) --------
    # xa: full Z^T, tiled per column-tile: [n][p][c][col], element = x[c*128+p, n*512+col]
    xa_d = nc.dram_tensor("xa", [NTILES, P, CTILES, NT], dt.bfloat16, kind="ExternalInput")
    # xm: my row block, transposed: [p][c][1024], element = x_mine[c*128+p, row]
    xm_d = nc.dram_tensor("xm", [P, CTILES, RPC], dt.bfloat16, kind="ExternalInput")
    # xmT/xpT: my / partner row blocks, row-major tiled: [p][i][d]
    xmT_d = nc.dram_tensor("xmT", [P, ITILES, D], dt.bfloat16, kind="ExternalInput")
    xpT_d = nc.dram_tensor("xpT", [P, ITILES, D], dt.bfloat16, kind="ExternalInput")

    out_d = nc.dram_tensor("loss_rows", [P, ITILES], dt.float32, kind="ExternalOutput")
    dbg_d = nc.dram_tensor("dbg", [P, ITILES, 4], dt.float32, kind="ExternalOutput")

    from contextlib import ExitStack

    with tile.TileContext(nc) as tc, ExitStack() as ctx:
        const = ctx.enter_context(tc.tile_pool(name="const", bufs=1))
        prol = ctx.enter_context(tc.tile_pool(name="prol", bufs=1))
        persist = ctx.enter_context(tc.tile_pool(name="persist", bufs=1))
        nstream = ctx.enter_context(tc.tile_pool(name="nstream", bufs=3))
        small = ctx.enter_context(tc.tile_pool(name="small", bufs=3))
        junkp = ctx.enter_context(tc.tile_pool(name="junk", bufs=4))
        psum_s = ctx.enter_context(tc.tile_pool(name="psum_s", bufs=2, space="PSUM"))
        psum_b = ctx.enter_context(tc.tile_pool(name="psum_b", bufs=2, space="PSUM"))
        psum_m = ctx.enter_context(tc.tile_pool(name="psum_m", bufs=4, space="PSUM"))

        ones_col = const.tile([P, 1], dt.bfloat16)
        nc.vector.memset(ones_col, 1.0)
        ones_row = const.tile([1, P], dt.bfloat16)
        nc.vector.memset(ones_row, 1.0)

        # ---------------- mine prep: z_mine (lhsT layout) ----------------
        xm_s = prol.tile([P, CTILES, RPC], dt.bfloat16)
        nc.gpsimd.dma_start(out=xm_s, in_=xm_d[:])
        sqm = prol.tile([P, CTILES, RPC], dt.bfloat16)
        nc.vector.tensor_mul(sqm, xm_s, xm_s)
        zm = persist.tile([P, CTILES, RPC], dt.bfloat16)
        for h in range(RPC // NT):  # 2 halves of 512 cols
            hs = slice(h * NT, (h + 1) * NT)
            ps = psum_s.tile([1, NT], dt.float32)
            for c in range(CTILES):
                nc.tensor.matmul(ps, ones_col, sqm[:, c, hs],
                                 start=(c == 0), stop=(c == CTILES - 1))
            ln_m = small.tile([1, NT], dt.bfloat16)
            nc.scalar.activation(ln_m, ps, AF.Ln)
            pb = psum_b.tile([P, NT], dt.float32)
            nc.tensor.matmul(pb, ones_row, ln_m)
            rb_m = small.tile([P, NT], dt.bfloat16)
            nc.scalar.activation(rb_m, pb, AF.Exp, scale=-0.5)
            rb_m_b = bass.AP(tensor=rb_m.tensor, offset=rb_m.offset,
                             ap=[rb_m.ap[0], [0, CTILES], rb_m.ap[1]])
            nc.vector.tensor_mul(zm[:, :, hs], xm_s[:, :, hs], rb_m_b)

        # ---------------- T-side inputs (compute deferred past main loop) ----
        xmT_s = prol.tile([P, ITILES, D], dt.bfloat16)
        nc.gpsimd.dma_start(out=xmT_s, in_=xmT_d[:])
        xpT_s = prol.tile([P, ITILES, D], dt.bfloat16)
        nc.gpsimd.dma_start(out=xpT_s, in_=xpT_d[:])

        # ---------------- main loop over column tiles ----------------
        # xa staged in one persistent tile: 16 DMAs into disjoint n-slices,
        # fresh memory each -> at most 1 sem wait per DMA (HW limit).
        # Norm chain for tile n+1 is emitted BEFORE tile n's main matmuls so
        # the PE's in-order stream never stalls on the ACT/DVE chain.
        xa_f = persist.tile([P, NTILES, CTILES, NT], dt.bfloat16)
        rowpart = persist.tile([P, ITILES, NTILES], dt.float32)

        def norm_tile(n):
            nc.gpsimd.dma_start(out=xa_f[:, n, :, :], in_=xa_d[n])
            sq_n = nstream.tile([P, CTILES, NT], dt.bfloat16, name=f"sq_{n}",
                                tag="sq_n", bufs=3)
            nc.vector.tensor_mul(sq_n, xa_f[:, n, :, :], xa_f[:, n, :, :])
            ps = psum_s.tile([1, NT], dt.float32, name=f"ps_{n}", tag="ps")
            for c in range(CTILES):
                nc.tensor.matmul(ps, ones_col, sq_n[:, c, :],
                                 start=(c == 0), stop=(c == CTILES - 1))
            ln_n = small.tile([1, NT], dt.bfloat16, name=f"ln_{n}", tag="ln_n")
            nc.scalar.activation(ln_n, ps, AF.Ln)
            pb = psum_b.tile([P, NT], dt.float32, name=f"pb_{n}", tag="pb")
            nc.tensor.matmul(pb, ones_row, ln_n)
            rb_n = small.tile([P, NT], dt.bfloat16, name=f"rb_{n}", tag="rb_n")
            nc.scalar.activation(rb_n, pb, AF.Exp, scale=-0.5)
            rb_n_b = bass.AP(tensor=rb_n.tensor, offset=rb_n.offset,
                             ap=[rb_n.ap[0], [0, CTILES], rb_n.ap[1]])
            za_n = nstream.tile([P, CTILES, NT], dt.bfloat16, name=f"za_{n}",
                                tag="za_n", bufs=3)
            nc.vector.tensor_mul(za_n, xa_f[:, n, :, :], rb_n_b)
            return za_n

        za_tiles = {0: norm_tile(0), 1: norm_tile(1)}
        for n in range(NTILES):
            if n + 2 < NTILES:
                za_tiles[n + 2] = norm_tile(n + 2)
            za_n = za_tiles.pop(n)
            for i in range(ITILES):
                pm = psum_m.tile([P, NT], dt.float32, name=f"pm_{n}_{i}", tag="pm")
                for c in range(CTILES):
                    nc.tensor.matmul(pm, zm[:, c, i * P:(i + 1) * P], za_n[:, c, :],
                                     start=(c == 0), stop=(c == CTILES - 1))
                ej = junkp.tile([P, NT], dt.bfloat16, name=f"ej_{n}_{i}", tag="ej")
                nc.scalar.activation(ej, pm, AF.Exp, scale=INV_T,
                                     accum_out=rowpart[:, i, n:n + 1])

        # ---------------- T-side compute: positives & self terms -------------
        tmp8 = prol.tile([P, ITILES, D], dt.bfloat16)
        ssq_m = small.tile([P, ITILES], dt.float32)
        nc.vector.tensor_mul(tmp8, xmT_s, xmT_s)
        nc.vector.tensor_reduce(ssq_m, tmp8, axis=mybir.AxisListType.X, op=ALU.add)
        ssq_p = small.tile([P, ITILES], dt.float32)
        nc.vector.tensor_mul(tmp8, xpT_s, xpT_s)
        nc.vector.tensor_reduce(ssq_p, tmp8, axis=mybir.AxisListType.X, op=ALU.add)
        r_m = small.tile([P, ITILES], dt.float32)
        r_p = small.tile([P, ITILES], dt.float32)
        nc.scalar.activation(r_m, ssq_m, AF.Ln)
        nc.scalar.activation(r_m, r_m, AF.Exp, scale=-0.5)
        nc.scalar.activation(r_p, ssq_p, AF.Ln)
        nc.scalar.activation(r_p, r_p, AF.Exp, scale=-0.5)
        zmT = prol.tile([P, ITILES, D], dt.bfloat16)
        zpT = prol.tile([P, ITILES, D], dt.bfloat16)
        for i in range(ITILES):
            nc.vector.tensor_scalar_mul(zmT[:, i, :], xmT_s[:, i, :], r_m[:, i:i + 1])
            nc.vector.tensor_scalar_mul(zpT[:, i, :], xpT_s[:, i, :], r_p[:, i:i + 1])
        posT = small.tile([P, ITILES], dt.float32)
        nc.vector.tensor_mul(tmp8, zmT, zpT)
        nc.vector.tensor_reduce(posT, tmp8, axis=mybir.AxisListType.X, op=ALU.add)
        s2T = small.tile([P, ITILES], dt.float32)
        nc.vector.tensor_mul(tmp8, zmT, zmT)
        nc.vector.tensor_reduce(s2T, tmp8, axis=mybir.AxisListType.X, op=ALU.add)

        # ---------------- epilogue: per-row losses ----------------
        rowsum = small.tile([P, ITILES], dt.float32)
        nc.vector.tensor_reduce(rowsum, rowpart, axis=mybir.AxisListType.X,
                                op=ALU.add)
        selfe = small.tile([P, ITILES], dt.float32)
        nc.scalar.activation(selfe, s2T, AF.Exp, scale=INV_T)
        denom = small.tile([P, ITILES], dt.float32)
        nc.vector.tensor_sub(denom, rowsum, selfe)
        ld = small.tile([P, ITILES], dt.float32)
        nc.scalar.activation(ld, denom, AF.Ln)
        negpos = small.tile([P, ITILES], dt.float32)
        nc.vector.tensor_scalar_mul(negpos, posT, -INV_T)
        loss_t = small.tile([P, ITILES], dt.float32)
        nc.vector.tensor_add(loss_t, ld, negpos)
        nc.gpsimd.dma_start(out=out_d[:], in_=loss_t)

        dbg = small.tile([P, ITILES, 4], dt.float32)
        nc.vector.tensor_copy(dbg[:, :, 0], rowsum)
        nc.vector.tensor_copy(dbg[:, :, 1], denom)
        nc.vector.tensor_copy(dbg[:, :, 2], posT)
        nc.vector.tensor_copy(dbg[:, :, 3], s2T)
        nc.gpsimd.dma_start(out=dbg_d[:], in_=dbg)

    nc.compile()
    return nc


def _get_nc():
    if "nc" not in _CACHE:
        _CACHE["nc"] = _build_bass()
    return _CACHE["nc"]


def _prep_inputs(polyline_embs, c_embs):
    """Host-side shard/tile prep. Returns in_maps for the 8 cores."""
    bf16 = ml_dtypes.bfloat16
    z = np.concatenate([np.asarray(polyline_embs, np.float32),
                        np.asarray(c_embs, np.float32)], axis=0)  # [8192, 512]
    zb = z.astype(bf16)                                            # quantize once

    # xa: [512, 8192]^T tiled -> [n][p][c][col]
    xt = np.ascontiguousarray(zb.T)                                # [512, 8192]
    xa = np.ascontiguousarray(
        xt.reshape(CTILES, P, NTILES, NT).transpose(2, 1, 0, 3))   # [16,128,4,512]

    in_maps = []
    for k in range(NCORES):
        rows = zb[k * RPC:(k + 1) * RPC]                           # [1024, 512]
        prows_start = (k * RPC + B) % N
        prows = zb[prows_start:prows_start + RPC]
        xm = np.ascontiguousarray(
            rows.T.reshape(CTILES, P, RPC).transpose(1, 0, 2))     # [128, 4, 1024]
        xmT = np.ascontiguousarray(
            rows.reshape(ITILES, P, D).transpose(1, 0, 2))         # [128, 8, 512]
        xpT = np.ascontiguousarray(
            prows.reshape(ITILES, P, D).transpose(1, 0, 2))        # [128, 8, 512]
        in_maps.append({"xa": xa, "xm": xm, "xmT": xmT, "xpT": xpT})
    return in_maps


def kernel(polyline_embs, c_embs):
    from concourse.bass_utils import run_bass_kernel_spmd

    nc = _get_nc()
    in_maps = _prep_inputs(polyline_embs, c_embs)
    res = run_bass_kernel_spmd(nc, in_maps, core_ids=list(range(NCORES)))
    _CACHE["last_results"] = res
    total = 0.0
    for r in res.results:
        total += r["loss_rows"].astype(np.float64).sum()
    return np.float32(total / N)



# revision 5
# speedup vs baseline: 2.0951x; 2.0951x over previous
"""InfoNCE loss kernel for Trainium2, 8 NeuronCores.

Reference computation:
    z = l2_normalize(concat([polyline_embs, c_embs]))   # [8192, 512]
    sim = z @ z.T                                       # [8192, 8192]
    denom_i = sum_{j != i} exp(sim_ij / T)
    pos_i   = sim[i, i +- B]
    loss    = mean(log(denom_i) - pos_i / T)

Sharding: row-block parallel. Core k computes rows [k*1024, (k+1)*1024) of sim
against all 8192 columns (the "all-gather" is free: every core receives the
full transposed embedding matrix as input). The scalar loss is reduced on host
(the "all-reduce").

Perf notes vs the first working version:
  - All activations forced into the one table set that has both exp and ln
    (natural_log_exp_and_others); the greedy per-instruction set choice was
    emitting 35 ACT_TABLE_LOADs (~45us on the bottleneck engine).
  - Matmul operands are fp8 e4m3 with perf_mode=DoubleRow (contraction 256
    per instruction): embeddings are scaled by ALPHA=16 on-device during
    normalization so typical |z| ~ 0.7 sits in e4m3's normal range; the exp
    scale compensates with 2/ALPHA^2.
  - exp ACTIVATEs batched 2 column-tiles wide over a 2-bank PSUM tile,
    halving the per-instruction overhead (~300 cycles) and the
    ACTIVATION_READ_ACCUMULATOR count.
"""

import math

import numpy as np
import ml_dtypes

B = 4096
D = 512
N = 2 * B            # 8192 rows of sim
NCORES = 8
RPC = N // NCORES    # 1024 rows per core
P = 128              # partitions
ITILES = RPC // P    # 8 row tiles per core
CTILES = D // P      # 4 contraction chunks of 128
NT = 512             # column-tile width
NTILES = N // NT     # 16 column tiles
NPAIRS = NTILES // 2  # 8 pairs of column tiles (exp batch unit)
INV_T = 2.0          # 1 / temperature
ALPHA = 16.0         # fp8 pre-scale; sim comes out scaled by ALPHA^2
LN_ALPHA = math.log(ALPHA)
EXP_SCALE = INV_T / (ALPHA * ALPHA)

_CACHE = {}


def _patch_activation_tables():
    """Force every activation onto the natural_log_exp_and_others table set.

    The greedy table-load insertion pass picks, per instruction, the first
    act_info.json set containing the function; alternating Exp/Ln picks two
    different sets and thrashes ACT_TABLE_LOAD (~1.3us each). Blanking all
    other sets (list order, hence act_func_set_id, preserved) makes the
    fixpoint hoist a single load. CoreSim reads the same patched view.
    """
    import functools
    from concourse import hw_specs, bacc, bass_interp

    if getattr(hw_specs.get_activation_tables, "_infonce_patched", False):
        return
    orig = hw_specs.get_activation_tables
    KEEP = "natural_log_exp_and_others"

    @functools.cache
    def patched(module_arch):
        tabs = orig(module_arch)
        return {k: (v if k == KEEP else set()) for k, v in tabs.items()}

    patched._infonce_patched = True
    hw_specs.get_activation_tables = patched
    bacc.get_activation_tables = patched
    bass_interp.get_activation_tables = patched


def _build_bass():
    """Trace the per-core Bass program (identical for all 8 cores)."""
    import concourse.bass as bass
    import concourse.tile as tile
    from concourse import bacc, mybir

    _patch_activation_tables()

    dt = mybir.dt
    AF = mybir.ActivationFunctionType
    ALU = mybir.AluOpType
    DR = mybir.MatmulPerfMode.DoubleRow

    nc = bacc.Bacc(None, target_bir_lowering=False, debug=False, num_swdge_queues=4)

    # -------- DRAM I/O (host-pretiled) --------
    # xa: full Z^T in fp8, tiled per column-tile: [n][p][c][col],
    # element = x[c*128+p, n*512+col]
    xa_d = nc.dram_tensor("xa", [NTILES, P, CTILES, NT], dt.float8e4, kind="ExternalInput")
    # xm: my row block, transposed: [p][c][1024], element = x_mine[c*128+p, row]
    xm_d = nc.dram_tensor("xm", [P, CTILES, RPC], dt.float8e4, kind="ExternalInput")
    # xmT/xpT: my / partner row blocks, row-major tiled: [p][i][d], bf16
    xmT_d = nc.dram_tensor("xmT", [P, ITILES, D], dt.bfloat16, kind="ExternalInput")
    xpT_d = nc.dram_tensor("xpT", [P, ITILES, D], dt.bfloat16, kind="ExternalInput")

    out_d = nc.dram_tensor("loss_rows", [P, ITILES], dt.float32, kind="ExternalOutput")
    dbg_d = nc.dram_tensor("dbg", [P, ITILES, 4], dt.float32, kind="ExternalOutput")

    from contextlib import ExitStack

    with tile.TileContext(nc) as tc, ExitStack() as ctx:
        const = ctx.enter_context(tc.tile_pool(name="const", bufs=1))
        prol = ctx.enter_context(tc.tile_pool(name="prol", bufs=1))
        persist = ctx.enter_context(tc.tile_pool(name="persist", bufs=1))
        nstream = ctx.enter_context(tc.tile_pool(name="nstream", bufs=3))
        small = ctx.enter_context(tc.tile_pool(name="small", bufs=3))
        junkp = ctx.enter_context(tc.tile_pool(name="junk", bufs=4))
        psum_s = ctx.enter_context(tc.tile_pool(name="psum_s", bufs=2, space="PSUM"))
        psum_b = ctx.enter_context(tc.tile_pool(name="psum_b", bufs=2, space="PSUM"))
        psum_m = ctx.enter_context(tc.tile_pool(name="psum_m", bufs=2, space="PSUM"))

        ones_col8 = const.tile([P, 1], dt.float8e4)
        nc.vector.memset(ones_col8, 1.0)
        ones_row = const.tile([1, P], dt.bfloat16)
        nc.vector.memset(ones_row, 1.0)
        lnalpha = const.tile([P, 1], dt.float32)
        nc.vector.memset(lnalpha, LN_ALPHA)

        # ---------------- mine prep: z_mine (lhsT layout, fp8) ----------------
        xm_s = prol.tile([P, CTILES, RPC], dt.float8e4)
        nc.gpsimd.dma_start(out=xm_s, in_=xm_d[:])
        sqm = prol.tile([P, CTILES, RPC], dt.float8e4)
        nc.vector.tensor_mul(sqm, xm_s, xm_s)
        zm = persist.tile([P, CTILES, RPC], dt.float8e4)
        for h in range(RPC // NT):  # 2 halves of 512 cols
            hs = slice(h * NT, (h + 1) * NT)
            ps = psum_s.tile([1, NT], dt.float32)
            for c in range(CTILES):
                nc.tensor.matmul(ps, ones_col8, sqm[:, c, hs],
                                 start=(c == 0), stop=(c == CTILES - 1))
            ln_m = small.tile([1, NT], dt.bfloat16)
            nc.scalar.activation(ln_m, ps, AF.Ln)
            pb = psum_b.tile([P, NT], dt.float32)
            nc.tensor.matmul(pb, ones_row, ln_m)
            rb_m = small.tile([P, NT], dt.bfloat16)
            nc.scalar.activation(rb_m, pb, AF.Exp, scale=-0.5, bias=lnalpha)
            rb_m_b = bass.AP(tensor=rb_m.tensor, offset=rb_m.offset,
                             ap=[rb_m.ap[0], [0, CTILES], rb_m.ap[1]])
            nc.vector.tensor_mul(zm[:, :, hs], xm_s[:, :, hs], rb_m_b)

        # ---------------- T-side inputs (compute deferred past main loop) ----
        xmT_s = prol.tile([P, ITILES, D], dt.bfloat16)
        nc.gpsimd.dma_start(out=xmT_s, in_=xmT_d[:])
        xpT_s = prol.tile([P, ITILES, D], dt.bfloat16)
        nc.gpsimd.dma_start(out=xpT_s, in_=xpT_d[:])

        # ---------------- main loop over column tiles ----------------
        # xa staged in one persistent tile: 16 DMAs into disjoint n-slices.
        # Norm chain for tile n+2 is emitted BEFORE tile n's main matmuls so
        # the PE's in-order stream never stalls on the ACT/DVE chain.
        xa_f = persist.tile([P, NTILES, CTILES, NT], dt.float8e4)
        rowpart = persist.tile([P, ITILES, NPAIRS], dt.float32)

        def norm_tile(n):
            nc.gpsimd.dma_start(out=xa_f[:, n, :, :], in_=xa_d[n])
            sq_n = nstream.tile([P, CTILES, NT], dt.float8e4, name=f"sq_{n}",
                                tag="sq_n", bufs=3)
            nc.vector.tensor_mul(sq_n, xa_f[:, n, :, :], xa_f[:, n, :, :])
            ps = psum_s.tile([1, NT], dt.float32, name=f"ps_{n}", tag="ps")
            for c in range(CTILES):
                nc.tensor.matmul(ps, ones_col8, sq_n[:, c, :],
                                 start=(c == 0), stop=(c == CTILES - 1))
            ln_n = small.tile([1, NT], dt.bfloat16, name=f"ln_{n}", tag="ln_n")
            nc.scalar.activation(ln_n, ps, AF.Ln)
            pb = psum_b.tile([P, NT], dt.float32, name=f"pb_{n}", tag="pb")
            nc.tensor.matmul(pb, ones_row, ln_n)
            rb_n = small.tile([P, NT], dt.bfloat16, name=f"rb_{n}", tag="rb_n")
            nc.scalar.activation(rb_n, pb, AF.Exp, scale=-0.5, bias=lnalpha)
            rb_n_b = bass.AP(tensor=rb_n.tensor, offset=rb_n.offset,
                             ap=[rb_n.ap[0], [0, CTILES], rb_n.ap[1]])
            za_n = nstream.tile([P, CTILES, NT], dt.float8e4, name=f"za_{n}",
                                tag="za_n", bufs=3)
            nc.vector.tensor_mul(za_n, xa_f[:, n, :, :], rb_n_b)
            return za_n

        za_tiles = {0: norm_tile(0), 1: norm_tile(1), 2: norm_tile(2)}
        for np_ in range(NPAIRS):
            n0, n1 = 2 * np_, 2 * np_ + 1
            for nn in (n0 + 3, n0 + 4):
                if nn < NTILES:
                    za_tiles[nn] = norm_tile(nn)
            za0 = za_tiles.pop(n0)
            za1 = za_tiles.pop(n1)
            for i in range(ITILES):
                pm = psum_m.tile([P, 2, NT], dt.float32, name=f"pm_{np_}_{i}",
                                 tag="pm")
                for half, za in ((0, za0), (1, za1)):
                    for cc in range(CTILES // 2):
                        nc.tensor.matmul(
                            pm[:, half, :],
                            zm[:, 2 * cc:2 * cc + 2, i * P:(i + 1) * P],
                            za[:, 2 * cc:2 * cc + 2, :],
                            start=(cc == 0), stop=(cc == CTILES // 2 - 1),
                            perf_mode=DR)
                ej = junkp.tile([P, 2, NT], dt.float8e4, name=f"ej_{np_}_{i}",
                                tag="ej")
                nc.scalar.activation(ej, pm, AF.Exp, scale=EXP_SCALE,
                                     accum_out=rowpart[:, i, np_:np_ + 1])

        # ---------------- T-side compute: positives & self terms -------------
        tmp8 = prol.tile([P, ITILES, D], dt.bfloat16)
        ssq_m = small.tile([P, ITILES], dt.float32)
        nc.vector.tensor_mul(tmp8, xmT_s, xmT_s)
        nc.vector.tensor_reduce(ssq_m, tmp8, axis=mybir.AxisListType.X, op=ALU.add)
        ssq_p = small.tile([P, ITILES], dt.float32)
        nc.vector.tensor_mul(tmp8, xpT_s, xpT_s)
        nc.vector.tensor_reduce(ssq_p, tmp8, axis=mybir.AxisListType.X, op=ALU.add)
        r_m = small.tile([P, ITILES], dt.float32)
        r_p = small.tile([P, ITILES], dt.float32)
        nc.scalar.activation(r_m, ssq_m, AF.Ln)
        nc.scalar.activation(r_m, r_m, AF.Exp, scale=-0.5)
        nc.scalar.activation(r_p, ssq_p, AF.Ln)
        nc.scalar.activation(r_p, r_p, AF.Exp, scale=-0.5)
        zmT = prol.tile([P, ITILES, D], dt.bfloat16)
        zpT = prol.tile([P, ITILES, D], dt.bfloat16)
        for i in range(ITILES):
            nc.vector.tensor_scalar_mul(zmT[:, i, :], xmT_s[:, i, :], r_m[:, i:i + 1])
            nc.vector.tensor_scalar_mul(zpT[:, i, :], xpT_s[:, i, :], r_p[:, i:i + 1])
        posT = small.tile([P, ITILES], dt.float32)
        nc.vector.tensor_mul(tmp8, zmT, zpT)
        nc.vector.tensor_reduce(posT, tmp8, axis=mybir.AxisListType.X, op=ALU.add)
        s2T = small.tile([P, ITILES], dt.float32)
        nc.vector.tensor_mul(tmp8, zmT, zmT)
        nc.vector.tensor_reduce(s2T, tmp8, axis=mybir.AxisListType.X, op=ALU.add)

        # ---------------- epilogue: per-row losses ----------------
        rowsum = small.tile([P, ITILES], dt.float32)
        nc.vector.tensor_reduce(rowsum, rowpart, axis=mybir.AxisListType.X,
                                op=ALU.add)
        selfe = small.tile([P, ITILES], dt.float32)
        nc.scalar.activation(selfe, s2T, AF.Exp, scale=INV_T)
        denom = small.tile([P, ITILES], dt.float32)
        nc.vector.tensor_sub(denom, rowsum, selfe)
        ld = small.tile([P, ITILES], dt.float32)
        nc.scalar.activation(ld, denom, AF.Ln)
        negpos = small.tile([P, ITILES], dt.float32)
        nc.vector.tensor_scalar_mul(negpos, posT, -INV_T)
        loss_t = small.tile([P, ITILES], dt.float32)
        nc.vector.tensor_add(loss_t, ld, negpos)
        nc.gpsimd.dma_start(out=out_d[:], in_=loss_t)

        dbg = small.tile([P, ITILES, 4], dt.float32)
        nc.vector.tensor_copy(dbg[:, :, 0], rowsum)
        nc.vector.tensor_copy(dbg[:, :, 1], denom)
        nc.vector.tensor_copy(dbg[:, :, 2], posT)
        nc.vector.tensor_copy(dbg[:, :, 3], s2T)
        nc.gpsimd.dma_start(out=dbg_d[:], in_=dbg)

    nc.compile()
    return nc


def _get_nc():
    if "nc" not in _CACHE:
        _CACHE["nc"] = _build_bass()
    return _CACHE["nc"]


def _prep_inputs(polyline_embs, c_embs):
    """Host-side shard/tile prep. Returns in_maps for the 8 cores."""
    bf16 = ml_dtypes.bfloat16
    fp8 = ml_dtypes.float8_e4m3
    z = np.concatenate([np.asarray(polyline_embs, np.float32),
                        np.asarray(c_embs, np.float32)], axis=0)  # [8192, 512]
    zb = z.astype(bf16)        # for the positives path
    z8 = z.astype(fp8)         # for the similarity matmul (quantize once)

    # xa: [512, 8192]^T tiled -> [n][p][c][col]
    xt = np.ascontiguousarray(z8.T)                                # [512, 8192]
    xa = np.ascontiguousarray(
        xt.reshape(CTILES, P, NTILES, NT).transpose(2, 1, 0, 3))   # [16,128,4,512]

    in_maps = []
    for k in range(NCORES):
        rows8 = z8[k * RPC:(k + 1) * RPC]                          # [1024, 512]
        rows = zb[k * RPC:(k + 1) * RPC]
        prows_start = (k * RPC + B) % N
        prows = zb[prows_start:prows_start + RPC]
        xm = np.ascontiguousarray(
            rows8.T.reshape(CTILES, P, RPC).transpose(1, 0, 2))    # [128, 4, 1024]
        xmT = np.ascontiguousarray(
            rows.reshape(ITILES, P, D).transpose(1, 0, 2))         # [128, 8, 512]
        xpT = np.ascontiguousarray(
            prows.reshape(ITILES, P, D).transpose(1, 0, 2))        # [128, 8, 512]
        in_maps.append({"xa": xa, "xm": xm, "xmT": xmT, "xpT": xpT})
    return in_maps


def kernel(polyline_embs, c_embs):
    from concourse.bass_utils import run_bass_kernel_spmd

    nc = _get_nc()
    in_maps = _prep_inputs(polyline_embs, c_embs)
    res = run_bass_kernel_spmd(nc, in_maps, core_ids=list(range(NCORES)))
    _CACHE["last_results"] = res
    total = 0.0
    for r in res.results:
        total += r["loss_rows"].astype(np.float64).sum()
    return np.float32(total / N)
